# revision 7
# baseline (speedup 1.0000x reference)
"""Trainium2 Bass kernel for nn_AdvancedQuantumLayer (B=64, n=16 qubits, depth=3).

The reference circuit is: per-qubit RY(x_q) state prep, then 3 layers of
[CX(0,1)..CX(14,15) chain, then RY(theta[d,q]) on every qubit], then
P(qubit 0 = 1).

Exact light-cone reduction: every CX in the chain has control i -> target
i+1, and the measured observable is Z on qubit 0. In the Heisenberg
picture the back-propagated observable's support grows toward higher
qubit indices by at most one qubit per layer (X_k -> X_k X_{k+1} under
CX(k,k+1) conjugation; Z_k is invariant on the control side). With
depth=3 the final support is qubits {0,1,2,3}, so the output depends
ONLY on x[:, :4] and thetas[:, :4], and equals the same circuit run on
4 qubits with the CX chain truncated to (0,1),(1,2),(2,3). Verified
exact to ~1e-15 in float64 against the full 16-qubit simulation.

Device computation per core (8 samples, data-parallel across 8 cores):
  1. u = (cos(x/2), sin(x/2)) via ScalarE Sin activation
  2. build the 16-dim product state psi0 via 6 tensor_scalar muls
  3. transpose (8,16)->(16,8) on TensorE
  4. psi' = M @ psi0 where M is the 16x16 total circuit matrix
     (theta-dependent weights, built host-side from 12 angles)
  5. p1 = sum over basis states with qubit0=1 of psi'^2, via a square
     and a masked-reduction matmul
"""

import numpy as np

import concourse.bacc as bacc
import concourse.bass as bass
import concourse.mybir as mybir
import concourse.tile as tile
from concourse.bass_utils import run_bass_kernel_spmd

N_CORES = 8
B = 64
PER = B // N_CORES  # samples per core
NQ = 4  # light-cone qubits
DIM = 1 << NQ  # 16
F32 = mybir.dt.float32


def _build_circuit_matrix(thetas: np.ndarray) -> np.ndarray:
    """Total 16x16 circuit matrix on qubits 0..3 (qubit 0 = MSB), float64."""

    def ry(t):
        c, s = np.cos(t / 2), np.sin(t / 2)
        return np.array([[c, -s], [s, c]], dtype=np.float64)

    def cx(q):  # CX(control=q, target=q+1) as a basis permutation
        P = np.zeros((DIM, DIM), dtype=np.float64)
        for i in range(DIM):
            ctrl = (i >> (NQ - 1 - q)) & 1
            j = i ^ (1 << (NQ - 2 - q)) if ctrl else i
            P[j, i] = 1.0
        return P

    I2 = np.eye(2, dtype=np.float64)
    M = np.eye(DIM, dtype=np.float64)
    for d in range(thetas.shape[0]):
        L = cx(0)
        L = cx(1) @ L
        L = cx(2) @ L
        for q in range(NQ):
            mats = [I2] * NQ
            mats[q] = ry(np.float64(thetas[d, q]))
            K = mats[0]
            for m in mats[1:]:
                K = np.kron(K, m)
            L = K @ L
        M = L @ M
    return M


_NC_CACHE = None


def _build_nc() -> bass.Bass:
    global _NC_CACHE
    if _NC_CACHE is not None:
        return _NC_CACHE

    nc = bacc.Bacc(None, target_bir_lowering=False)
    xs = nc.dram_tensor("xs", [PER, NQ], F32, kind="ExternalInput")
    mt = nc.dram_tensor("mt", [DIM, DIM], F32, kind="ExternalInput")
    ident = nc.dram_tensor("ident", [PER, PER], F32, kind="ExternalInput")
    mask = nc.dram_tensor("mask", [DIM, 1], F32, kind="ExternalInput")
    out = nc.dram_tensor("out", [PER, 1], F32, kind="ExternalOutput")

    SIN = mybir.ActivationFunctionType.Sin

    with tile.TileContext(nc) as tc:
        with (
            tc.tile_pool(name="sbuf", bufs=1) as pool,
            tc.tile_pool(name="psum", bufs=1, space="PSUM") as psum,
        ):
            xt = pool.tile([PER, NQ], F32)
            nc.sync.dma_start(xt[:], xs[:])
            mtt = pool.tile([DIM, DIM], F32)
            nc.sync.dma_start(mtt[:], mt[:])
            idt = pool.tile([PER, PER], F32)
            nc.sync.dma_start(idt[:], ident[:])
            mkt = pool.tile([DIM, 1], F32)
            nc.sync.dma_start(mkt[:], mask[:])

            # u = (cos(x/2), sin(x/2)); cos via sin(x/2 + pi/2)
            xb = pool.tile([PER, NQ], F32)
            nc.vector.tensor_scalar(
                xb[:],
                xt[:],
                0.5,
                float(np.pi / 2),
                op0=mybir.AluOpType.mult,
                op1=mybir.AluOpType.add,
            )
            uc = pool.tile([PER, NQ], F32)
            us = pool.tile([PER, NQ], F32)
            nc.scalar.activation(uc[:], xb[:], SIN)
            nc.scalar.activation(us[:], xt[:], SIN, scale=0.5)

            # product state: psi[b, i] = prod_q u_q[b, bit_q(i)], qubit 0 = MSB
            psiA = pool.tile([PER, DIM], F32)
            psiB = pool.tile([PER, DIM], F32)
            nc.vector.tensor_copy(psiA[:, 0:1], uc[:, 3:4])
            nc.vector.tensor_copy(psiA[:, 1:2], us[:, 3:4])
            cur, nxt, m = psiA, psiB, 2
            for q in (2, 1, 0):
                nc.vector.tensor_scalar_mul(nxt[:, 0:m], cur[:, 0:m], uc[:, q : q + 1])
                nc.vector.tensor_scalar_mul(
                    nxt[:, m : 2 * m], cur[:, 0:m], us[:, q : q + 1]
                )
                cur, nxt = nxt, cur
                m *= 2

            # (PER, DIM) -> (DIM, PER)
            pT = psum.tile([DIM, PER], F32)
            nc.tensor.transpose(pT[:], cur[:], idt[:])
            sT = pool.tile([DIM, PER], F32)
            nc.vector.tensor_copy(sT[:], pT[:])

            # psi' = M @ psi0: lhsT[j, i] = M[i, j]
            pC = psum.tile([DIM, PER], F32)
            nc.tensor.matmul(pC[:], mtt[:], sT[:], start=True, stop=True)
            sC = pool.tile([DIM, PER], F32)
            nc.vector.tensor_copy(sC[:], pC[:])
            sq = pool.tile([DIM, PER], F32)
            nc.vector.tensor_mul(sq[:], sC[:], sC[:])

            # p1[b] = sum_i mask[i] * sq[i, b]
            pO = psum.tile([PER, 1], F32)
            nc.tensor.matmul(pO[:], sq[:], mkt[:], start=True, stop=True)
            so = pool.tile([PER, 1], F32)
            nc.vector.tensor_copy(so[:], pO[:])
            nc.sync.dma_start(out[:], so[:])

    if not nc.is_finalized():
        nc.finalize()
    _NC_CACHE = nc
    return nc


def _run(x: np.ndarray, thetas: np.ndarray, **spmd_kwargs):
    x = np.ascontiguousarray(np.asarray(x, dtype=np.float32))
    thetas = np.asarray(thetas, dtype=np.float32)
    assert x.shape == (B, 16) and thetas.shape[1] == 16

    M = _build_circuit_matrix(thetas[:, :NQ].astype(np.float64))
    mt = np.ascontiguousarray(M.T.astype(np.float32))
    ident = np.eye(PER, dtype=np.float32)
    mask = np.zeros((DIM, 1), dtype=np.float32)
    mask[DIM // 2 :, 0] = 1.0  # basis states with qubit 0 == 1

    in_maps = []
    for c in range(N_CORES):
        shard = np.ascontiguousarray(x[c * PER : (c + 1) * PER, :NQ])
        in_maps.append({"xs": shard, "mt": mt, "ident": ident, "mask": mask})

    nc = _build_nc()
    res = run_bass_kernel_spmd(nc, in_maps, core_ids=list(range(N_CORES)), **spmd_kwargs)
    outs = np.concatenate([r["out"] for r in res.results], axis=0)
    return outs.astype(np.float32), res


def kernel(x: np.ndarray, thetas: np.ndarray) -> np.ndarray:
    outs, _ = _run(x, thetas)
    return outs


# revision 21
# speedup vs baseline: 1.2605x; 1.2605x over previous
"""Trainium2 Bass kernel for nn_AdvancedQuantumLayer (B=64, n=16 qubits, depth=3).

The reference circuit is: per-qubit RY(x_q) state prep, then 3 layers of
[CX(0,1)..CX(14,15) chain, then RY(theta[d,q]) on every qubit], then
P(qubit 0 = 1).

Exact light-cone reduction: every CX in the chain has control i -> target
i+1, and the measured observable is Z on qubit 0. In the Heisenberg
picture the back-propagated observable's support grows toward higher
qubit indices by at most one qubit per layer (X_k -> X_k X_{k+1} under
CX(k,k+1) conjugation; Z_k is invariant on the control side). With
depth=3 the final support is qubits {0,1,2,3}, so the output depends
ONLY on x[:, :4] and thetas[:, :4], and equals the same circuit run on
4 qubits with the CX chain truncated to (0,1),(1,2),(2,3). Verified
exact to ~1e-15 in float64 against the full 16-qubit simulation.

Device computation per core (8 samples, data-parallel across 8 cores):
  1. one DMA of a packed (16,30) tensor: M^T | mask | I8 | x-shard
  2. sin(x/2) = x*P(x^2), cos(x/2) = Q(x^2) as Horner chains on the
     vector engine (no ScalarE activation: the piecewise-poly activation
     tables are loaded by an NRT-patched DMA whose completion is not
     synchronized with the first ACTIVATE on a cold NEFF - measured
     first-execution failures with stale/empty Sin tables)
  3. build the 16-dim product state psi0 via 6 tensor_scalar muls (DVE)
  4. transpose (8,16)->(16,8) on TensorE
  5. psi' = M @ psi0 where M is the 16x16 total circuit matrix
     (theta-dependent weights, built host-side from 12 angles)
  6. p1 = mask-weighted column sums of psi'^2 via one more matmul
  7. DMA the (8,1) result back to HBM

Raw Bacc (no TileContext) with 3 manual semaphores; cross-engine
hand-offs increment semaphores from an explicit engine drain so
consumers cannot observe in-flight writes.
"""

import numpy as np

import concourse.bacc as bacc
import concourse.bass as bass
import concourse.mybir as mybir
from concourse.bass_utils import run_bass_kernel_spmd

N_CORES = 8
B = 64
PER = B // N_CORES  # samples per core
NQ = 4  # light-cone qubits
DIM = 1 << NQ  # 16
CW = 30  # packed const tensor width
F32 = mybir.dt.float32

# minimax-ish (Chebyshev) fits on x in [-6, 6]:
#   sin(x/2) = x * P(x^2), max abs err 3.9e-9
#   cos(x/2) = Q(x^2),     max abs err 1.9e-10
_SIN_P = [
    4.9999999873562655e-01,
    -2.0833330216259132e-02,
    2.6041444949660264e-04,
    -1.5566750362944743e-06,
    5.1706788878085897e-09,
    -1.0684670042033123e-11,
    1.1703142574745029e-14,
]
_COS_Q = [
    9.9999999997461426e-01,
    -1.2499999950212243e-01,
    2.6041662586081832e-03,
    -2.1700969729235996e-05,
    9.6029862245036837e-08,
    -2.5631494633173576e-10,
    4.3701563208650786e-13,
    -4.2482954002717051e-16,
]


def _fit_polys():
    """Recompute the fits (sanity check / reproducibility helper)."""
    xs = np.linspace(-6, 6, 20001)
    z = xs**2
    f = np.where(xs == 0, 0.5, np.sin(xs / 2) / np.where(xs == 0, 1, xs))
    P = (
        np.polynomial.chebyshev.Chebyshev.fit(z, f, 6, domain=[0, 36])
        .convert(kind=np.polynomial.Polynomial)
        .coef
    )
    Q = (
        np.polynomial.chebyshev.Chebyshev.fit(z, np.cos(xs / 2), 7, domain=[0, 36])
        .convert(kind=np.polynomial.Polynomial)
        .coef
    )
    return P, Q


def _build_circuit_matrix(thetas: np.ndarray) -> np.ndarray:
    """Total 16x16 circuit matrix on qubits 0..3 (qubit 0 = MSB), float64."""

    def ry(t):
        c, s = np.cos(t / 2), np.sin(t / 2)
        return np.array([[c, -s], [s, c]], dtype=np.float64)

    def cx(q):  # CX(control=q, target=q+1) as a basis permutation
        P = np.zeros((DIM, DIM), dtype=np.float64)
        for i in range(DIM):
            ctrl = (i >> (NQ - 1 - q)) & 1
            j = i ^ (1 << (NQ - 2 - q)) if ctrl else i
            P[j, i] = 1.0
        return P

    I2 = np.eye(2, dtype=np.float64)
    M = np.eye(DIM, dtype=np.float64)
    for d in range(thetas.shape[0]):
        L = cx(0)
        L = cx(1) @ L
        L = cx(2) @ L
        for q in range(NQ):
            mats = [I2] * NQ
            mats[q] = ry(np.float64(thetas[d, q]))
            K = mats[0]
            for m in mats[1:]:
                K = np.kron(K, m)
            L = K @ L
        M = L @ M
    return M


def _pack_consts(M: np.ndarray, x_shard: np.ndarray) -> np.ndarray:
    """One (16, CW) tensor: cols 0:16 M^T, col 16 mask, cols 17:25 I8,
    cols 25:29 x shard (rows 0:8)."""
    C = np.zeros((DIM, CW), dtype=np.float32)
    C[:, 0:DIM] = M.T.astype(np.float32)
    C[DIM // 2 :, DIM] = 1.0  # mask: basis states with qubit 0 == 1
    C[0:PER, 17 : 17 + PER] = np.eye(PER, dtype=np.float32)
    C[0:PER, 25:29] = x_shard
    return C


_NC_CACHE = None


def _build_nc() -> bass.Bass:
    global _NC_CACHE
    if _NC_CACHE is not None:
        return _NC_CACHE

    nc = bacc.Bacc(None, target_bir_lowering=False)
    cin = nc.dram_tensor("cin", [DIM, CW], F32, kind="ExternalInput")
    out = nc.dram_tensor("out", [PER, 1], F32, kind="ExternalOutput")

    MULT = mybir.AluOpType.mult
    ADD = mybir.AluOpType.add

    with (
        nc.sbuf_tensor([DIM, CW], F32) as ct,
        nc.sbuf_tensor([PER, NQ], F32) as zz,
        nc.sbuf_tensor([PER, 2 * NQ], F32) as wk,
        nc.sbuf_tensor([PER, 2 * NQ], F32) as uus,  # [cos(x/2) | sin(x/2)]
        nc.sbuf_tensor([PER, DIM], F32) as psiA,
        nc.sbuf_tensor([PER, DIM], F32) as psiB,
        nc.sbuf_tensor([DIM, PER], F32) as sT,
        nc.sbuf_tensor([DIM, PER], F32) as sC,
        nc.sbuf_tensor([DIM, PER], F32) as sq,
        nc.sbuf_tensor([PER, 1], F32) as so,
        nc.psum_tensor([DIM, PER], F32) as pT,
        nc.psum_tensor([DIM, PER], F32) as pC,
        nc.psum_tensor([PER, 1], F32) as pO,
        nc.semaphore() as dma_sem,
        nc.semaphore() as sV,
        nc.semaphore() as sP,
        nc.Block() as block,
    ):
        mtt = ct[:, 0:DIM]  # lhsT: mtt[j, i] = M[i, j]
        mkt = ct[:, DIM : DIM + 1]  # mask column
        idt = ct[0:PER, 17 : 17 + PER]  # identity for transpose
        xt = ct[0:PER, 25:29]  # x shard
        cosv = uus[:, 0:NQ]
        sinv = uus[:, NQ : 2 * NQ]

        @block.sync
        def _(sync):
            sync.dma_start(ct[:], cin[:]).then_inc(dma_sem, 16)
            sync.wait_ge(sV, 2)
            sync.dma_start(out[:], so[:]).then_inc(dma_sem, 16)
            # block-end barrier must not retire before the output lands in HBM
            sync.wait_ge(dma_sem, 32)

        @block.vector
        def _(vector):
            v = nc.vector
            vector.wait_ge(dma_sem, 16)
            # sin(x/2) = x*P(z), cos(x/2) = Q(z), z = x^2, Estrin form.
            # DVE has a same-engine RAW hazard only between ADJACENT
            # dependent ops; the chains are interleaved so every consumer
            # is >= 2 slots after its producer, with drains at the two
            # points where that cannot be arranged.
            As, Bs, Cs = wk[:, 0:4], wk[:, 4:8], wk[:, 8:12]
            Ac, Bc, Cc = wk[:, 12:16], wk[:, 16:20], wk[:, 20:24]
            Dc, t1, s2, s3 = wk[:, 24:28], wk[:, 28:32], wk[:, 32:36], wk[:, 36:40]
            s4, c2, c3, c4 = wk[:, 40:44], zz[:], pd[:], wk[:, 44:48]
            c5 = wk[:, 48:52]
            v.tensor_mul(zz[:], xt, xt)
            vector.drain()
            v.tensor_mul(z2[:], zz[:], zz[:])
            v.tensor_scalar(As, zz[:], _SIN_P[1], _SIN_P[0], op0=MULT, op1=ADD)
            v.tensor_scalar(Ac, zz[:], _COS_Q[1], _COS_Q[0], op0=MULT, op1=ADD)
            v.tensor_scalar(Bs, zz[:], _SIN_P[3], _SIN_P[2], op0=MULT, op1=ADD)
            v.tensor_scalar(Bc, zz[:], _COS_Q[3], _COS_Q[2], op0=MULT, op1=ADD)
            v.tensor_scalar(Cs, zz[:], _SIN_P[5], _SIN_P[4], op0=MULT, op1=ADD)
            v.tensor_scalar(Cc, zz[:], _COS_Q[5], _COS_Q[4], op0=MULT, op1=ADD)
            v.tensor_scalar_mul(Dc, z2[:], _COS_Q[6])
            v.tensor_mul(t1, Cs, z2[:])
            v.tensor_add(c2, Cc, Dc)
            v.tensor_add(s2, Bs, t1)
            v.tensor_mul(c3, c2, z2[:])
            v.tensor_mul(s3, s2, z2[:])
            v.tensor_add(c4, Bc, c3)
            v.tensor_add(s4, As, s3)
            v.tensor_mul(c5, c4, z2[:])
            v.tensor_mul(sinv, s4, xt)
            v.tensor_add(cosv, Ac, c5)
            v.tensor_copy(t1, z2[:])  # spacer: cosv (prev op) -> builds gap
            # product state psi0[b, i] = prod_q u_q[b, bit_q(i)], qubit 0 = MSB
            v1 = uus[:, 3 : 2 * NQ : NQ]  # [cos3 | sin3] stride-4 pair
            v.tensor_scalar_mul(psiA[:, 0:2], v1, uus[:, 2:3])
            v.tensor_scalar_mul(psiA[:, 2:4], v1, uus[:, 6:7])
            v.tensor_copy(s2, z2[:])  # spacer: b2 -> b3 gap
            v.tensor_scalar_mul(psiB[:, 0:4], psiA[:, 0:4], uus[:, 1:2])
            v.tensor_scalar_mul(psiB[:, 4:8], psiA[:, 0:4], uus[:, 5:6])
            v.tensor_copy(s3, z2[:])  # spacer: b4 -> b5 gap
            v.tensor_scalar_mul(psiA[:, 0:8], psiB[:, 0:8], uus[:, 0:1])
            v.tensor_scalar_mul(psiA[:, 8:16], psiB[:, 0:8], uus[:, 4:5])
            vector.drain().then_inc(sP, 1)
            vector.wait_ge(sP, 2)  # transpose done
            v.tensor_copy(sT[:], pT[:])
            vector.drain().then_inc(sP, 1)
            vector.wait_ge(sP, 4)  # circuit matmul done
            v.tensor_copy(sC[:, 0:4], pC[:, 0:4])
            v.tensor_copy(sC[:, 4:8], pC[:, 4:8])
            v.tensor_mul(sq[:, 0:4], sC[:, 0:4], sC[:, 0:4])
            v.tensor_mul(sq[:, 4:8], sC[:, 4:8], sC[:, 4:8])
            vector.drain().then_inc(sP, 1)
            vector.wait_ge(sP, 6)  # reduction matmul done
            v.tensor_copy(so[:], pO[:])
            vector.drain().then_inc(sV, 2)

        @block.tensor
        def _(tensor):
            tensor.wait_ge(sP, 1)
            nc.tensor.transpose(pT[:], psiA[:], idt).then_inc(sP, 1)
            tensor.wait_ge(sP, 3)
            nc.tensor.matmul(pC[:], mtt, sT[:], start=True, stop=True).then_inc(sP, 1)
            tensor.wait_ge(sP, 5)
            nc.tensor.matmul(pO[:], sq[:], mkt, start=True, stop=True).then_inc(sP, 1)

    # The const-AP memsets emitted by Bass.__init__ (0.0/1.0 fp32, bf16,
    # uint8 consts) are dead code here - no instruction reads them - but
    # they sit at the head of the measured window. Drop them.
    main_bb = nc.main_func.blocks[0]
    main_bb.instructions = [
        ins
        for ins in main_bb.instructions
        if not (
            type(ins).__name__ == "InstMemset"
            and any(
                getattr(o, "memsetref", "").startswith("const-")
                or "const-" in str(getattr(o, "memref", ""))
                for o in ins.outs
            )
        )
    ]

    if not nc.is_finalized():
        nc.finalize()
    _NC_CACHE = nc
    return nc


def _run(x: np.ndarray, thetas: np.ndarray, **spmd_kwargs):
    x = np.asarray(x, dtype=np.float32)
    thetas = np.asarray(thetas, dtype=np.float32)
    assert x.shape == (B, 16) and thetas.shape[1] == 16

    M = _build_circuit_matrix(thetas[:, :NQ].astype(np.float64))
    in_maps = [
        {"cin": _pack_consts(M, x[c * PER : (c + 1) * PER, :NQ])}
        for c in range(N_CORES)
    ]

    nc = _build_nc()
    res = run_bass_kernel_spmd(nc, in_maps, core_ids=list(range(N_CORES)), **spmd_kwargs)
    outs = np.concatenate([r["out"] for r in res.results], axis=0)
    return outs.astype(np.float32), res


def kernel(x: np.ndarray, thetas: np.ndarray) -> np.ndarray:
    outs, _ = _run(x, thetas)
    return outs
